# revision 18
# baseline (speedup 1.0000x reference)
"""COMPASSNet MoE-routing kernel for 8 TRN2 NeuronCores.

Problem: B=262144 samples of D=32 features with NaNs at 0/1/2 positions;
each of P=529 NaN patterns owns a tiny MLP (32 -> 4 -> 1, tanh/sigmoid).
y[b] = sigmoid(W2[p].tanh(x0[b] @ W1[p] + b1[p]) + b2[p]), p = pattern id.

Sharding strategy (host side, part of constructing per-core shards):
samples are grouped by pattern (stable sort of pattern_ids), patterns are
greedy bin-packed across the 8 cores, and each pattern group is padded to
a multiple of 128 sample slots.  All per-pattern parameters are folded
into dense per-tile operand streams so the device kernel is a fully
static, branch-free pipeline at the memory roofline.

Device kernel (SPMD, identical program on all 8 cores):
  - A "tile" = 512 sample slots packed 4-per-PE-column: the stationary
    matmul operand X4[t] is (K=128 = 4 slots x 32 features, M=128
    columns).  The moving operand is a (128, 16) block-diagonal weight
    matrix (slot s rows 32s..32s+31, cols 4s..4s+3 hold W1[pattern of
    slot s]).  One PE matmul per 512 samples -> h_pre in PSUM with
    samples on partitions.
  - b1 / W2 / b2 are applied with rank-1 (K=1 ones-column) matmuls that
    broadcast host-prepared per-tile rows across all 128 partitions.
  - tanh on ACT, H*W2 multiply + segment-sum(4) on DVE, + b2, sigmoid,
    DMA out.  Output order is unscrambled on the host.
"""

import ml_dtypes
import numpy as np

import concourse.bass as bass
import concourse.tile as tile
from concourse import mybir
from concourse.bass_utils import run_bass_kernel_spmd

F32 = mybir.dt.float32
BF16 = mybir.dt.bfloat16
MM_DT = BF16          # dtype of the big matmul operand streams
MM_NP = ml_dtypes.bfloat16 if MM_DT == BF16 else np.float32

B = 262144
D = 32
P = 529
H = 4
H5 = 5          # hidden + ones column (b2 folded into W2)
N_CORES = 8
SLOT = 128          # pattern groups padded to multiples of this
TILE = 512          # samples per PE stationary tile (4 slots x 128 cols)
import os
MT_MAX = int(os.environ.get("KMT", "25"))  # tiles per megatile (<=25: 25*20 <= 512 f32 PSUM bank)


# ----------------------------------------------------------------- host pack
def _pack(x, pattern_ids, W1, b1, W2, b2):
    """Build per-core device operand streams. Returns (T, in_maps, scatter)."""
    pid = np.asarray(pattern_ids).astype(np.int64).ravel()
    x = np.asarray(x, dtype=np.float32)
    W1 = np.asarray(W1, dtype=np.float32)
    b1 = np.asarray(b1, dtype=np.float32)
    W2 = np.asarray(W2, dtype=np.float32)
    b2 = np.asarray(b2, dtype=np.float32)

    order = np.argsort(pid, kind="stable")
    counts = np.bincount(pid, minlength=P)
    starts = np.zeros(P + 1, np.int64)
    np.cumsum(counts, out=starts[1:])

    # greedy bin-pack patterns over cores by 128-slot units
    units = (counts + SLOT - 1) // SLOT          # slot units per pattern
    pat_order = np.argsort(-counts, kind="stable")
    core_units = np.zeros(N_CORES, np.int64)
    core_pats = [[] for _ in range(N_CORES)]
    for p in pat_order:
        c = int(np.argmin(core_units))
        core_pats[c].append(int(p))
        core_units[c] += units[p]
    T = int((core_units.max() * SLOT + TILE - 1) // TILE)

    S = T * TILE
    in_maps = []
    scatter = []                                  # (orig_indices, packed_pos)
    ones_row = np.ones(1, np.float32)
    for c in range(N_CORES):
        idx = np.full(S, -1, np.int64)            # packed slot -> orig sample
        slot_pat = np.zeros(T * 4, np.int64)      # 128-slot block -> pattern
        pos = 0
        for p in core_pats[c]:
            n = int(counts[p])
            if n:
                idx[pos:pos + n] = order[starts[p]:starts[p] + n]
            nblk = (n + SLOT - 1) // SLOT
            slot_pat[pos // SLOT: pos // SLOT + nblk] = p
            pos += nblk * SLOT
        valid = idx >= 0
        x0 = np.zeros((S, D), np.float32)
        xv = x[idx[valid]]
        np.nan_to_num(xv, copy=False)
        x0[valid] = xv

        # X4r[p=32s+d, t, m] = x0[t*512 + s*128 + m, d]
        X4 = x0.reshape(T, 4, SLOT, D).transpose(0, 1, 3, 2).reshape(T, 128, 128)
        X4r = np.ascontiguousarray(X4.transpose(1, 0, 2)).astype(MM_NP)

        sp = slot_pat.reshape(T, 4)
        # block-diagonal W1 per tile, hidden extended to H5=5: the 5th
        # column is 0 in W1, 20.0 in b1 (tanh(20) == 1.0f), and b2 in W2 —
        # so layer 2's bias rides the weighted reduce for free.
        W1e = np.zeros((P, D, H5), np.float32)
        W1e[:, :, :H] = W1
        b1e = np.full((P, H5), 20.0, np.float32)
        b1e[:, :H] = b1
        W2e = np.zeros((P, H5), np.float32)
        W2e[:, :H] = W2
        W2e[:, H] = b2
        WB = np.zeros((T, 4, D, 4, H5), np.float32)
        s4 = np.arange(4)
        WB[:, s4, :, s4, :] = W1e[sp].transpose(1, 0, 2, 3)
        WBr = np.ascontiguousarray(
            WB.reshape(T, 128, 4 * H5).transpose(1, 0, 2)).astype(MM_NP)

        BR = b1e[sp].reshape(1, T * 4 * H5)
        W2R = W2e[sp].reshape(1, T * 4 * H5)

        in_maps.append({
            "x4": X4r, "wb": WBr,
            "br": np.ascontiguousarray(BR).astype(MM_NP),
            "w2r": np.ascontiguousarray(W2R).astype(MM_NP),
        })
        scatter.append((idx, valid))
    return T, in_maps, scatter


# ------------------------------------------------------------- device build
def _split_excess_waits(nc, cap=1):
    """walrus here rejects >1 sync wait per instruction; move extras onto
    same-engine NoOps placed immediately before the owner."""
    f = nc.m.functions[0]
    for bb in list(f.blocks):
        out, changed = [], False
        for inst in bb.instructions:
            si = inst.sync_info
            waits = list(si.on_wait) if si is not None else []
            if len(waits) > cap:
                for w in waits[:-cap]:
                    out.append(mybir.InstNoOp(
                        name=nc.get_next_instruction_name(),
                        sync_info=mybir.SyncInfo(on_wait=[w], on_update=[]),
                        bass_nofuse=True,
                        engine=inst.engine,
                    ))
                si.on_wait = waits[-cap:]
                changed = True
            out.append(inst)
        if changed:
            bb.instructions = out


def _build(T):
    nc = bass.Bass("TRN2", target_bir_lowering=False, debug=False)
    x4 = nc.declare_dram_parameter("x4", [128, T, 128], MM_DT, isOutput=False)
    wb = nc.declare_dram_parameter("wb", [128, T, 4 * H5], MM_DT, isOutput=False)
    br = nc.declare_dram_parameter("br", [1, T * 4 * H5], MM_DT, isOutput=False)
    w2r = nc.declare_dram_parameter("w2r", [1, T * 4 * H5], MM_DT, isOutput=False)
    y = nc.declare_dram_parameter("y", [128, T * 4], F32, isOutput=True)

    # big chunks first, small trailing chunks so the final post-op chain
    # (which serializes after the last input DMA) is short
    mts = []
    t = T
    while t > 8:
        mts.append(min(MT_MAX, t - 8))
        t -= mts[-1]
    while t > 0:
        mts.append(min(8, t))
        t -= mts[-1]

    with tile.TileContext(nc) as tc:
        with (
            tc.tile_pool(name="consts", bufs=1) as consts,
            tc.tile_pool(name="xp", bufs=4) as xp,
            tc.tile_pool(name="wp", bufs=4) as wp,
            tc.tile_pool(name="hp", bufs=3) as hp,
            tc.tile_pool(name="op", bufs=3) as op,
            tc.tile_pool(name="ps1", bufs=4, space="PSUM") as ps1p,
            tc.tile_pool(name="ps2", bufs=2, space="PSUM") as ps2p,
        ):
            ones = consts.tile([1, 128], MM_DT)
            nc.vector.memset(ones, 1.0)
            br_sb = consts.tile([1, T * 4 * H5], MM_DT)
            w2_sb = consts.tile([1, T * 4 * H5], MM_DT)
            y_sb = consts.tile([128, T * 4], F32)

            t0 = 0
            did_half = False
            half_cols = 0
            for mi, mt in enumerate(mts):
                g = mt * 4                      # output cols this megatile
                xt = xp.tile([128, mt, 128], MM_DT)
                nc.sync.dma_start(out=xt, in_=x4[:, t0:t0 + mt, :])
                wt = wp.tile([128, mt, 4 * H5], MM_DT)
                nc.sync.dma_start(out=wt, in_=wb[:, t0:t0 + mt, :])
                if mi == 0:
                    nc.sync.dma_start(out=br_sb, in_=br[:, :])
                    nc.sync.dma_start(out=w2_sb, in_=w2r[:, :])

                ps1 = ps1p.tile([128, g, H5], F32)
                for tt in range(mt):
                    nc.tensor.matmul(
                        out=ps1[:, tt * 4:(tt + 1) * 4, :],
                        lhsT=xt[:, tt, :],
                        rhs=wt[:, tt, :],
                        # start=True resets has_written for the whole PSUM
                        # bank, so only the first matmul per bank may set it
                        start=(tt == 0), stop=False,
                    )
                # += b1 broadcast (rank-1: ones-column x bias row)
                nc.tensor.matmul(
                    out=ps1[:, :, :],
                    lhsT=ones,
                    rhs=br_sb[:, t0 * 4 * H5:(t0 + mt) * 4 * H5],
                    start=False, stop=True,
                )
                # [W2 | b2] broadcast tile
                ps2 = ps2p.tile([128, g, H5], F32)
                nc.tensor.matmul(
                    out=ps2, lhsT=ones,
                    rhs=w2_sb[:, t0 * 4 * H5:(t0 + mt) * 4 * H5],
                    start=True, stop=True,
                )

                ht = hp.tile([128, g, H5], F32)
                nc.scalar.activation(
                    out=ht, in_=ps1, func=mybir.ActivationFunctionType.Tanh)
                m2 = hp.tile([128, g, H5], F32)
                nc.vector.tensor_mul(m2, ht, ps2)
                gs = op.tile([128, g], F32)
                nc.vector.tensor_reduce(
                    out=gs, in_=m2, axis=mybir.AxisListType.X,
                    op=mybir.AluOpType.add)
                nc.scalar.activation(
                    out=y_sb[:, t0 * 4:t0 * 4 + g], in_=gs,
                    func=mybir.ActivationFunctionType.Sigmoid)
                t0 += mt
                if t0 * 2 >= T and not did_half:
                    nc.sync.dma_start(
                        out=y[:, :t0 * 4], in_=y_sb[:, :t0 * 4])
                    half_cols = t0 * 4
                    did_half = True
            nc.sync.dma_start(
                out=y[:, half_cols:], in_=y_sb[:, half_cols:])

    _split_excess_waits(nc)
    return nc


# ------------------------------------------------------------------- driver
def _run(inputs, trace=False):
    T, in_maps, scatter = _pack(**inputs)
    nc = _build(T)
    res = run_bass_kernel_spmd(
        nc, in_maps, core_ids=list(range(N_CORES)), trace=trace)
    out = np.zeros((B, 1), np.float32)
    for c in range(N_CORES):
        ydev = res.results[c]["y"]                # (128, T*4)
        ypack = np.ascontiguousarray(ydev.T).ravel()  # packed slot order
        idx, valid = scatter[c]
        out[idx[valid], 0] = ypack[valid]
    return out, res


def kernel(**inputs):
    out, _ = _run(inputs, trace=False)
    return out
